# revision 21
# baseline (speedup 1.0000x reference)
"""Multi-head causal attention (B=2, T=2048, D=1024, H=16) on 8 trn2 NeuronCores.

Sharding: 8 cores = 2 batches x 4 head-groups (4 heads each). Each core:
  - computes qkv projections for its 4 heads from x[b] (pre-transposed on host),
  - runs masked softmax attention in transposed (k, q) score layout,
  - emits a partial output projection y_part = attn_heads @ w_out[head_rows].
Host sums the 4 partial y per batch.

All matmuls run in bf16 with fp32 PSUM accumulation. Softmax skips the
max-subtraction (scores are ~N(0,1)): exp on ScalarE, row sums via an
appended ones-column on v, normalization via gpsimd partition_broadcast.
Mask handling is generic: the host classifies (128k x 512q) blocks of the
provided mask into skip / full / partial; partial blocks are multiplied in.
"""
import sys
sys.path.insert(0, "/opt/trn_rl_repo")

import numpy as np
import ml_dtypes

import concourse.bass as bass
import concourse.mybir as mybir
import concourse.tile as tile
from concourse import bacc
from concourse.bass_utils import run_bass_kernel_spmd

B, T, D, H, Dh = 2, 2048, 1024, 16, 64
P = 128
QT = 512              # q-tile width (score tile free dim)
NQ = T // QT          # 4
NKT = T // P          # 16
ND = D // P           # 8
HPC = 4               # heads per core
NPAIR = HPC // 2      # head pairs per core
N_CORES = 8

f32 = mybir.dt.float32
bf16 = mybir.dt.bfloat16
CDT = bf16            # compute dtype for matmul operands
NP_CDT = ml_dtypes.bfloat16


def _block_structure(mask: np.ndarray):
    """Classify maskT (k,q) blocks: per q-tile a list of (kt, pattern_idx|None).

    For each unique partial pattern also derive (w0, m_lo, m_hi): w0 leading
    all-masked columns (exp skipped, memset 0), and [m_lo, m_hi) the column
    range that still needs the mask multiply.
    """
    maskT = (mask != 0).T.astype(np.float32)  # [k, q] visibility
    vis = []
    patterns = []
    meta = []
    pat_index = {}
    for qt in range(NQ):
        row = []
        for kt in range(NKT):
            blk = maskT[kt * P:(kt + 1) * P, qt * QT:(qt + 1) * QT]
            s = blk.sum()
            if s == 0:
                continue
            if s == blk.size:
                row.append((kt, None))
            else:
                key = blk.tobytes()
                if key not in pat_index:
                    pat_index[key] = len(patterns)
                    patterns.append(blk)
                    col_any = blk.any(axis=0)       # column has any visible
                    col_all = blk.all(axis=0)       # column fully visible
                    w0 = int(np.argmax(col_any)) if col_any.any() else QT
                    partial_cols = np.nonzero(col_any & ~col_all)[0]
                    if partial_cols.size:
                        m_lo, m_hi = int(partial_cols[0]), int(partial_cols[-1]) + 1
                    else:
                        m_lo = m_hi = 0
                    meta.append((w0, m_lo, m_hi))
                row.append((kt, pat_index[key]))
        vis.append(row)
    if patterns:
        pm = np.stack(patterns)
    else:
        pm = np.zeros((1, P, QT), np.float32)
    return vis, pm, meta


def _build_program(vis, n_pm, meta=(), compile=True):
    nc = bacc.Bacc() if compile else bass.Bass()
    xT = nc.declare_dram_parameter("xT", [D, T], CDT, isOutput=False)
    wq = nc.declare_dram_parameter("wq", [D, HPC * Dh], CDT, isOutput=False)
    wk = nc.declare_dram_parameter("wk", [D, HPC * Dh], CDT, isOutput=False)
    wv = nc.declare_dram_parameter("wv", [D, HPC * Dh], CDT, isOutput=False)
    wo = nc.declare_dram_parameter("wo", [HPC * Dh, D], CDT, isOutput=False)
    pmask = nc.declare_dram_parameter("pmask", [n_pm, P, QT], CDT, isOutput=False)
    y = nc.declare_dram_parameter("y", [T, D], f32, isOutput=True)

    with tile.TileContext(nc) as tc:
        with (
            tc.tile_pool(name="persist", bufs=1) as persist,
            tc.tile_pool(name="work", bufs=3) as work,
            tc.tile_pool(name="psA", bufs=2, space="PSUM") as psA,
            tc.tile_pool(name="psS", bufs=1, space="PSUM") as psS,
            tc.tile_pool(name="psU", bufs=2, space="PSUM") as psU,
        ):
            # ---- persistent SBUF tensors ----
            xt_sb = persist.tile([P, ND, T], CDT, tag="xt")
            wq_sb = persist.tile([P, ND, HPC * Dh], CDT, tag="wq")
            wk_sb = persist.tile([P, ND, HPC * Dh], CDT, tag="wk")
            wv_sb = persist.tile([P, ND, HPC * Dh], CDT, tag="wv")
            wo_sb = persist.tile([P, NPAIR, D], CDT, tag="wo")
            pm_sb = persist.tile([P, n_pm, QT], CDT, tag="pm")
            qT_sb = persist.tile([P, NPAIR, T], CDT, tag="qT")
            kT_sb = persist.tile([P, NPAIR, T], CDT, tag="kT")
            # v1: per k-tile and head, [128, 128]: for even heads cols 0:64 =
            # v values and cols 64:128 all-ones (for odd heads the reverse),
            # so the attnU matmul emits softmax denominators replicated on the
            # complementary partition half (matmul cost only depends on N).
            v1_sb = persist.tile([P, NKT, HPC, P], CDT, tag="v1")
            at_sb = persist.tile([P, NPAIR, T], CDT, tag="at")

            nc.sync.dma_start(xt_sb[:], xT.rearrange("(o p) t -> p o t", p=P))
            nc.sync.dma_start(wq_sb[:], wq.rearrange("(o p) e -> p o e", p=P))
            nc.sync.dma_start(wk_sb[:], wk.rearrange("(o p) e -> p o e", p=P))
            nc.sync.dma_start(wv_sb[:], wv.rearrange("(o p) e -> p o e", p=P))
            nc.sync.dma_start(wo_sb[:], wo.rearrange("(o p) e -> p o e", p=P))
            nc.sync.dma_start(pm_sb[:], pmask.rearrange("n p q -> p n q"))

            # ---- phase A: v = x @ wv ----
            nc.vector.memset(v1_sb[:], 1.0)  # ones blocks; v halves overwritten below
            for tt in range(NKT):
                ps_v = psA.tile([P, QT], f32, tag="psA", name=f"psv{tt}")
                for dt in range(ND):
                    nc.tensor.matmul(
                        ps_v[:, :HPC * Dh],
                        xt_sb[:, dt, tt * P:(tt + 1) * P],
                        wv_sb[:, dt, :],
                        start=(dt == 0),
                        stop=(dt == ND - 1),
                    )
                ps_vh = ps_v[:, :HPC * Dh].rearrange("p (h e) -> p h e", h=HPC)
                nc.vector.tensor_copy(v1_sb[:, tt, 0::2, 0:Dh], ps_vh[:, 0::2])
                nc.vector.tensor_copy(v1_sb[:, tt, 1::2, Dh:P], ps_vh[:, 1::2])

            # ---- phase A: qT / kT pairs ----
            for w_sb, out_sb in ((wq_sb, qT_sb), (wk_sb, kT_sb)):
                for p in range(NPAIR):
                    for nt in range(NQ):
                        ps_qk = psA.tile([P, QT], f32, tag="psA", name=f"psqk{p}_{nt}")
                        for dt in range(ND):
                            nc.tensor.matmul(
                                ps_qk[:],
                                w_sb[:, dt, p * P:(p + 1) * P],
                                xt_sb[:, dt, nt * QT:(nt + 1) * QT],
                                start=(dt == 0),
                                stop=(dt == ND - 1),
                            )
                        nc.vector.tensor_copy(out_sb[:, p, nt * QT:(nt + 1) * QT], ps_qk[:])

            # ---- phase B: attention per (pair, q-tile) ----
            # Software-pipelined: scores/exp for step j+1 are emitted before
            # the attnU matmuls of step j so PE has independent work while
            # ScalarE computes exp.
            inv_sqrt_dh = 1.0 / float(np.sqrt(Dh))
            for p in range(NPAIR):
                for qt in range(NQ):
                    row = vis[qt]
                    if not row:
                        for h in range(2):
                            nc.vector.memset(
                                at_sb[h * Dh:(h + 1) * Dh, p, qt * QT:(qt + 1) * QT], 0.0)
                        continue
                    ps_u = [
                        psU.tile([P, QT], f32, tag=f"u{h}", name=f"u{h}_{p}_{qt}")
                        for h in range(2)
                    ]
                    es_q = []

                    def emit_scores(j):
                        kt, pidx = row[j]
                        pair_es = []
                        for h in range(2):
                            base = h * Dh
                            ps_s = psS.tile([P, QT], f32, tag=f"s{h}", name=f"s{h}_{p}_{qt}_{kt}")
                            nc.tensor.matmul(
                                ps_s[:],
                                kT_sb[base:base + Dh, p, kt * P:(kt + 1) * P],
                                qT_sb[base:base + Dh, p, qt * QT:(qt + 1) * QT],
                                start=True,
                                stop=True,
                            )
                            es = work.tile([P, QT], CDT, tag=f"es{h}", name=f"es{h}_{p}_{qt}_{kt}")
                            if pidx is None:
                                nc.scalar.activation(
                                    es[:], ps_s[:],
                                    mybir.ActivationFunctionType.Exp,
                                    scale=inv_sqrt_dh,
                                )
                            else:
                                w0, m_lo, m_hi = meta[pidx]
                                if w0 > 0:
                                    nc.vector.memset(es[:, 0:w0], 0.0)
                                nc.scalar.activation(
                                    es[:, w0:QT], ps_s[:, w0:QT],
                                    mybir.ActivationFunctionType.Exp,
                                    scale=inv_sqrt_dh,
                                )
                                if m_hi > m_lo:
                                    nc.vector.tensor_mul(
                                        es[:, m_lo:m_hi], es[:, m_lo:m_hi],
                                        pm_sb[:, pidx, m_lo:m_hi],
                                    )
                            pair_es.append(es)
                        es_q.append(pair_es)

                    def emit_attnu(j):
                        kt, _ = row[j]
                        pair_es = es_q[j]
                        for h in range(2):
                            nc.tensor.matmul(
                                ps_u[h],
                                v1_sb[:, kt, 2 * p + h, :],
                                pair_es[h][:],
                                start=(j == 0),
                                stop=(j == len(row) - 1),
                            )

                    emit_scores(0)
                    for j in range(len(row)):
                        if j + 1 < len(row):
                            emit_scores(j + 1)
                        emit_attnu(j)

                    for h in range(2):
                        # ps_u[h]: for h==0 partitions 0:64 = attnU, 64:128 =
                        # denominators (replicated); for h==1 the reverse.
                        # DVE lanes are partition-fixed, so shift the recips
                        # to the attn half with a small SBUF->SBUF DMA.
                        a_sl = slice(0, Dh) if h == 0 else slice(Dh, P)
                        s_sl = slice(Dh, P) if h == 0 else slice(0, Dh)
                        rep = work.tile([P, QT], f32, tag="rep", name=f"rep{h}_{p}_{qt}")
                        nc.vector.reciprocal(rep[s_sl, :], ps_u[h][s_sl, :])
                        rep2 = work.tile([P, QT], f32, tag="rep2", name=f"rep2{h}_{p}_{qt}")
                        nc.sync.dma_start(rep2[a_sl, :], rep[s_sl, :])
                        nc.vector.tensor_mul(
                            at_sb[h * Dh:(h + 1) * Dh, p, qt * QT:(qt + 1) * QT],
                            ps_u[h][a_sl, :],
                            rep2[a_sl, :],
                        )

            # ---- phase C: partial out-projection ----
            for tt in range(NKT):
                for half in range(2):
                    ps_y = psA.tile([P, QT], f32, tag="psA", name=f"psy{tt}_{half}")
                    for p in range(NPAIR):
                        nc.tensor.matmul(
                            ps_y[:],
                            at_sb[:, p, tt * P:(tt + 1) * P],
                            wo_sb[:, p, half * QT:(half + 1) * QT],
                            start=(p == 0),
                            stop=(p == NPAIR - 1),
                        )
                    ysb = work.tile([P, QT], f32, tag="y", name=f"y{tt}_{half}")
                    nc.vector.tensor_copy(ysb[:], ps_y[:])
                    nc.sync.dma_start(y[tt * P:(tt + 1) * P, half * QT:(half + 1) * QT], ysb[:])
    if compile:
        nc.compile()
    return nc


def _host_inputs(x, mask, w_qkv, w_out):
    vis, pm, meta = _block_structure(np.asarray(mask))
    pm_c = pm.astype(NP_CDT)
    wq_f, wk_f, wv_f = np.split(np.asarray(w_qkv, np.float32), 3, axis=1)
    in_maps = []
    for core in range(N_CORES):
        b = core // 4
        g = core % 4
        cols = slice(g * HPC * Dh, (g + 1) * HPC * Dh)
        in_maps.append({
            "xT": np.ascontiguousarray(np.asarray(x[b], np.float32).T).astype(NP_CDT),
            "wq": wq_f[:, cols].astype(NP_CDT),
            "wk": wk_f[:, cols].astype(NP_CDT),
            "wv": wv_f[:, cols].astype(NP_CDT),
            "wo": np.asarray(w_out, np.float32)[cols, :].astype(NP_CDT),
            "pmask": pm_c,
        })
    return vis, pm, meta, in_maps


def run(x, mask, w_qkv, w_out, trace=False):
    vis, pm, meta, in_maps = _host_inputs(x, mask, w_qkv, w_out)
    nc = _build_program(vis, pm.shape[0], meta)
    res = run_bass_kernel_spmd(nc, in_maps, core_ids=list(range(N_CORES)), trace=trace)
    parts = [res.results[i]["y"].astype(np.float32) for i in range(N_CORES)]
    out = np.stack([
        parts[0] + parts[1] + parts[2] + parts[3],
        parts[4] + parts[5] + parts[6] + parts[7],
    ]).astype(np.float32)
    return out, res


def kernel(x, mask, w_qkv, w_out):
    out, _ = run(x, mask, w_qkv, w_out, trace=False)
    return out


# revision 24
# speedup vs baseline: 1.3904x; 1.3904x over previous
"""Multi-head causal attention (B=2, T=2048, D=1024, H=16) on 8 trn2 NeuronCores.

Sharding: 8 cores = 2 batches x 4 head-groups (4 heads each). Each core:
  - computes qkv projections for its 4 heads from x[b] (pre-transposed on host),
  - runs masked softmax attention in transposed (k, q) score layout,
  - emits a partial output projection y_part = attn_heads @ w_out[head_rows].
Host sums the 4 partial y per batch.

All matmuls run in bf16 with fp32 PSUM accumulation. Softmax skips the
max-subtraction (scores are ~N(0,1)): exp on ScalarE, row sums via an
appended ones-column on v, normalization via gpsimd partition_broadcast.
Mask handling is generic: the host classifies (128k x 512q) blocks of the
provided mask into skip / full / partial; partial blocks are multiplied in.
"""
import sys
sys.path.insert(0, "/opt/trn_rl_repo")

import numpy as np
import ml_dtypes

import concourse.bass as bass
import concourse.mybir as mybir
import concourse.tile as tile
from concourse import bacc
from concourse.bass_utils import run_bass_kernel_spmd

B, T, D, H, Dh = 2, 2048, 1024, 16, 64
P = 128
QT = 512              # q-tile width (score tile free dim)
NQ = T // QT          # 4
NKT = T // P          # 16
ND = D // P           # 8
HPC = 4               # heads per core
NPAIR = HPC // 2      # head pairs per core
N_CORES = 8

f32 = mybir.dt.float32
bf16 = mybir.dt.bfloat16
CDT = bf16            # compute dtype for matmul operands
NP_CDT = ml_dtypes.bfloat16


def _block_structure(mask: np.ndarray):
    """Classify maskT (k,q) blocks: per q-tile a list of (kt, pattern_idx|None).

    For each unique partial pattern also derive (w0, m_lo, m_hi): w0 leading
    all-masked columns (exp skipped, memset 0), and [m_lo, m_hi) the column
    range that still needs the mask multiply.
    """
    maskT = (mask != 0).T.astype(np.float32)  # [k, q] visibility
    vis = []
    patterns = []
    meta = []
    pat_index = {}
    for qt in range(NQ):
        row = []
        for kt in range(NKT):
            blk = maskT[kt * P:(kt + 1) * P, qt * QT:(qt + 1) * QT]
            s = blk.sum()
            if s == 0:
                continue
            if s == blk.size:
                row.append((kt, None))
            else:
                key = blk.tobytes()
                if key not in pat_index:
                    pat_index[key] = len(patterns)
                    patterns.append(blk)
                    col_any = blk.any(axis=0)       # column has any visible
                    col_all = blk.all(axis=0)       # column fully visible
                    w0 = int(np.argmax(col_any)) if col_any.any() else QT
                    partial_cols = np.nonzero(col_any & ~col_all)[0]
                    if partial_cols.size:
                        m_lo, m_hi = int(partial_cols[0]), int(partial_cols[-1]) + 1
                    else:
                        m_lo = m_hi = 0
                    meta.append((w0, m_lo, m_hi))
                row.append((kt, pat_index[key]))
        vis.append(row)
    if patterns:
        pm = np.stack(patterns)
    else:
        pm = np.zeros((1, P, QT), np.float32)
    return vis, pm, meta


def _build_program(vis, n_pm, meta=(), compile=True):
    nc = bacc.Bacc() if compile else bass.Bass()
    xT = nc.declare_dram_parameter("xT", [D, T], CDT, isOutput=False)
    wq = nc.declare_dram_parameter("wq", [D, HPC * Dh], CDT, isOutput=False)
    wk = nc.declare_dram_parameter("wk", [D, HPC * Dh], CDT, isOutput=False)
    wv = nc.declare_dram_parameter("wv", [D, HPC * Dh], CDT, isOutput=False)
    wo = nc.declare_dram_parameter("wo", [HPC * Dh, D], CDT, isOutput=False)
    pmask = nc.declare_dram_parameter("pmask", [n_pm, P, QT], CDT, isOutput=False)
    y = nc.declare_dram_parameter("y", [T, D], f32, isOutput=True)

    with tile.TileContext(nc) as tc:
        with (
            tc.tile_pool(name="persist", bufs=1) as persist,
            tc.tile_pool(name="work", bufs=3) as work,
            tc.tile_pool(name="psA", bufs=2, space="PSUM") as psA,
            tc.tile_pool(name="psS", bufs=1, space="PSUM") as psS,
            tc.tile_pool(name="psU", bufs=2, space="PSUM") as psU,
        ):
            # ---- persistent SBUF tensors ----
            xt_sb = persist.tile([P, ND, T], CDT, tag="xt")
            wq_sb = persist.tile([P, ND, HPC * Dh], CDT, tag="wq")
            wk_sb = persist.tile([P, ND, HPC * Dh], CDT, tag="wk")
            wv_sb = persist.tile([P, ND, HPC * Dh], CDT, tag="wv")
            wo_sb = persist.tile([P, NPAIR, D], CDT, tag="wo")
            pm_sb = persist.tile([P, n_pm, QT], CDT, tag="pm")
            qT_sb = persist.tile([P, NPAIR, T], CDT, tag="qT")
            kT_sb = persist.tile([P, NPAIR, T], CDT, tag="kT")
            # v1: per k-tile and head, [128, 128]: for even heads cols 0:64 =
            # v values and cols 64:128 all-ones (for odd heads the reverse),
            # so the attnU matmul emits softmax denominators replicated on the
            # complementary partition half (matmul cost only depends on N).
            v1_sb = persist.tile([P, NKT, HPC, P], CDT, tag="v1")
            at_sb = persist.tile([P, NPAIR, T], CDT, tag="at")

            nc.sync.dma_start(xt_sb[:], xT.rearrange("(o p) t -> p o t", p=P))
            nc.sync.dma_start(wq_sb[:], wq.rearrange("(o p) e -> p o e", p=P))
            nc.sync.dma_start(wk_sb[:], wk.rearrange("(o p) e -> p o e", p=P))
            nc.sync.dma_start(wv_sb[:], wv.rearrange("(o p) e -> p o e", p=P))
            nc.sync.dma_start(wo_sb[:], wo.rearrange("(o p) e -> p o e", p=P))
            nc.sync.dma_start(pm_sb[:], pmask.rearrange("n p q -> p n q"))

            # ---- phase A: v = x @ wv ----
            nc.vector.memset(v1_sb[:], 1.0)  # ones blocks; v halves overwritten below
            for tt in range(NKT):
                ps_v = psA.tile([P, QT], f32, tag="psA", name=f"psv{tt}")
                for dt in range(ND):
                    nc.tensor.matmul(
                        ps_v[:, :HPC * Dh],
                        xt_sb[:, dt, tt * P:(tt + 1) * P],
                        wv_sb[:, dt, :],
                        start=(dt == 0),
                        stop=(dt == ND - 1),
                    )
                ps_vh = ps_v[:, :HPC * Dh].rearrange("p (h e) -> p h e", h=HPC)
                nc.vector.tensor_copy(v1_sb[:, tt, 0::2, 0:Dh], ps_vh[:, 0::2])
                nc.vector.tensor_copy(v1_sb[:, tt, 1::2, Dh:P], ps_vh[:, 1::2])

            # ---- phase A: qT / kT pairs ----
            for w_sb, out_sb in ((wq_sb, qT_sb), (wk_sb, kT_sb)):
                for p in range(NPAIR):
                    for nt in range(NQ):
                        ps_qk = psA.tile([P, QT], f32, tag="psA", name=f"psqk{p}_{nt}")
                        for dt in range(ND):
                            nc.tensor.matmul(
                                ps_qk[:],
                                w_sb[:, dt, p * P:(p + 1) * P],
                                xt_sb[:, dt, nt * QT:(nt + 1) * QT],
                                start=(dt == 0),
                                stop=(dt == ND - 1),
                            )
                        nc.vector.tensor_copy(out_sb[:, p, nt * QT:(nt + 1) * QT], ps_qk[:])

            # ---- phase B: attention per (pair, q-tile) ----
            # Software-pipelined: scores/exp for step j+1 are emitted before
            # the attnU matmuls of step j so PE has independent work while
            # ScalarE computes exp.
            inv_sqrt_dh = 1.0 / float(np.sqrt(Dh))
            for p in range(NPAIR):
                for qt in range(NQ):
                    row = vis[qt]
                    if not row:
                        for h in range(2):
                            nc.vector.memset(
                                at_sb[h * Dh:(h + 1) * Dh, p, qt * QT:(qt + 1) * QT], 0.0)
                        continue
                    ps_u = [
                        psU.tile([P, QT], f32, tag=f"u{h}", name=f"u{h}_{p}_{qt}")
                        for h in range(2)
                    ]
                    es_q = []

                    def emit_scores(j):
                        kt, pidx = row[j]
                        pair_es = []
                        for h in range(2):
                            base = h * Dh
                            ps_s = psS.tile([P, QT], f32, tag=f"s{h}", name=f"s{h}_{p}_{qt}_{kt}")
                            nc.tensor.matmul(
                                ps_s[:],
                                kT_sb[base:base + Dh, p, kt * P:(kt + 1) * P],
                                qT_sb[base:base + Dh, p, qt * QT:(qt + 1) * QT],
                                start=True,
                                stop=True,
                            )
                            es = work.tile([P, QT], CDT, tag=f"es{h}", name=f"es{h}_{p}_{qt}_{kt}")
                            if pidx is None:
                                nc.scalar.activation(
                                    es[:], ps_s[:],
                                    mybir.ActivationFunctionType.Exp,
                                    scale=inv_sqrt_dh,
                                )
                            else:
                                w0, m_lo, m_hi = meta[pidx]
                                if w0 > 0:
                                    nc.vector.memset(es[:, 0:w0], 0.0)
                                nc.scalar.activation(
                                    es[:, w0:QT], ps_s[:, w0:QT],
                                    mybir.ActivationFunctionType.Exp,
                                    scale=inv_sqrt_dh,
                                )
                                if m_hi > m_lo:
                                    nc.vector.tensor_mul(
                                        es[:, m_lo:m_hi], es[:, m_lo:m_hi],
                                        pm_sb[:, pidx, m_lo:m_hi],
                                    )
                            pair_es.append(es)
                        es_q.append(pair_es)

                    def emit_attnu(j):
                        kt, _ = row[j]
                        pair_es = es_q[j]
                        for h in range(2):
                            nc.tensor.matmul(
                                ps_u[h],
                                v1_sb[:, kt, 2 * p + h, :],
                                pair_es[h][:],
                                start=(j == 0),
                                stop=(j == len(row) - 1),
                            )

                    emit_scores(0)
                    for j in range(len(row)):
                        if j + 1 < len(row):
                            emit_scores(j + 1)
                        emit_attnu(j)

                    for h in range(2):
                        # ps_u[h]: for h==0 partitions 0:64 = attnU, 64:128 =
                        # denominators (replicated); for h==1 the reverse.
                        # DVE lanes are partition-fixed, so shift the recips
                        # to the attn half with a small SBUF->SBUF DMA.
                        # The custom-DVE reciprocal only works at base
                        # partition 0 on HW, so stage the denominators there.
                        a_sl = slice(0, Dh) if h == 0 else slice(Dh, P)
                        s_sl = slice(Dh, P) if h == 0 else slice(0, Dh)
                        sums = work.tile([P, QT], f32, tag="sums", name=f"sums{h}_{p}_{qt}")
                        nc.vector.tensor_copy(sums[s_sl, :], ps_u[h][s_sl, :])
                        if h == 0:
                            sums2 = work.tile([P, QT], f32, tag="sums2", name=f"sums2{h}_{p}_{qt}")
                            nc.sync.dma_start(sums2[0:Dh, :], sums[Dh:P, :])
                            rsrc = sums2
                        else:
                            rsrc = sums
                        rep = work.tile([P, QT], f32, tag="rep", name=f"rep{h}_{p}_{qt}")
                        nc.vector.reciprocal_approx_fast(rep[0:Dh, :], rsrc[0:Dh, :])
                        if h == 0:
                            mul_in1 = rep[0:Dh, :]
                        else:
                            rep2 = work.tile([P, QT], f32, tag="rep2", name=f"rep2{h}_{p}_{qt}")
                            nc.sync.dma_start(rep2[Dh:P, :], rep[0:Dh, :])
                            mul_in1 = rep2[Dh:P, :]
                        nc.vector.tensor_mul(
                            at_sb[h * Dh:(h + 1) * Dh, p, qt * QT:(qt + 1) * QT],
                            ps_u[h][a_sl, :],
                            mul_in1,
                        )

            # ---- phase C: partial out-projection ----
            for tt in range(NKT):
                for half in range(2):
                    ps_y = psA.tile([P, QT], f32, tag="psA", name=f"psy{tt}_{half}")
                    for p in range(NPAIR):
                        nc.tensor.matmul(
                            ps_y[:],
                            at_sb[:, p, tt * P:(tt + 1) * P],
                            wo_sb[:, p, half * QT:(half + 1) * QT],
                            start=(p == 0),
                            stop=(p == NPAIR - 1),
                        )
                    ysb = work.tile([P, QT], f32, tag="y", name=f"y{tt}_{half}")
                    nc.vector.tensor_copy(ysb[:], ps_y[:])
                    nc.sync.dma_start(y[tt * P:(tt + 1) * P, half * QT:(half + 1) * QT], ysb[:])
    if compile:
        nc.compile()
    return nc


def _host_inputs(x, mask, w_qkv, w_out):
    vis, pm, meta = _block_structure(np.asarray(mask))
    pm_c = pm.astype(NP_CDT)
    wq_f, wk_f, wv_f = np.split(np.asarray(w_qkv, np.float32), 3, axis=1)
    in_maps = []
    for core in range(N_CORES):
        b = core // 4
        g = core % 4
        cols = slice(g * HPC * Dh, (g + 1) * HPC * Dh)
        in_maps.append({
            "xT": np.ascontiguousarray(np.asarray(x[b], np.float32).T).astype(NP_CDT),
            "wq": wq_f[:, cols].astype(NP_CDT),
            "wk": wk_f[:, cols].astype(NP_CDT),
            "wv": wv_f[:, cols].astype(NP_CDT),
            "wo": np.asarray(w_out, np.float32)[cols, :].astype(NP_CDT),
            "pmask": pm_c,
        })
    return vis, pm, meta, in_maps


def run(x, mask, w_qkv, w_out, trace=False):
    vis, pm, meta, in_maps = _host_inputs(x, mask, w_qkv, w_out)
    nc = _build_program(vis, pm.shape[0], meta)
    res = run_bass_kernel_spmd(nc, in_maps, core_ids=list(range(N_CORES)), trace=trace)
    parts = [res.results[i]["y"].astype(np.float32) for i in range(N_CORES)]
    out = np.stack([
        parts[0] + parts[1] + parts[2] + parts[3],
        parts[4] + parts[5] + parts[6] + parts[7],
    ]).astype(np.float32)
    return out, res


def kernel(x, mask, w_qkv, w_out):
    out, _ = run(x, mask, w_qkv, w_out, trace=False)
    return out
